# revision 10
# baseline (speedup 1.0000x reference)
"""
DLI loss kernel for Trainium2 (8 NeuronCores, pure data parallel over batch).

Math
----
The reference computes, per (b, j) window pair:
    logits[b,j,k] = h_last[b,j]@w_h + cterm[b,k] + fc_b
    loss_pair     = LSE_k(logits masked to k in [j+3, len_b)) - logits[b,j,j+3]
The h_last@w_h and fc_b terms are constant in k, so they cancel exactly
between the LSE and the positive logit.  The whole LSTM drops out and

    loss = sum_{b, s in [3, len_b)} [ log(sum_{k=s}^{len_b-1} e^{cterm[b,k]})
                                      - cterm[b,s] ] / sum_b (len_b - 3)
    cterm[b,k] = encoder_output[b,k,:] @ fc_w[0, H:]   (valid region only)

cterm values are O(+-2) so no max-subtraction is needed for a stable exp.

Device pipeline (per core, 16 batch rows)
-----------------------------------------
  - enc arrives through SWDGE (gpsimd) cast-DMAs that convert fp32 -> bf16
    in the DMA datapath: no on-chip cast pass.  2-row chunks; partition p
    holds rows (2b, 2b+1), t in {4p..4p+3} (2KB HBM lines).  The gpsimd
    queue carries ONLY these triggers, so descriptor generation starts at
    the top of the kernel.
  - All constants are precomputed on the host and DMA'd on the Scalar ring:
    identity (for PE transpose), the one-hot matvec weights
    woh[e, 16b+m] = w[e]*(m==b), and mask3 (f32 mask with first 3 cols
    zeroed).  The denominator sum(len_b - 3) is computed on the host.
  - PE: bf16 warm-up matmuls (HAM un-throttle), then per row 4 bf16
    transposes -> PSUM and a bf16 one-hot matvec (row b of cterm[16,512],
    PSUM-accumulated).  Transposes are LDWEIGHTS-bound (~107ns each).
  - PSUM->SBUF copies of transposed rows split across DVE and ACT; matvec
    for row b is emitted with skew 3 so it never head-blocks the PE queue.
  - Tail: exp (ACT; tables pre-warmed Ln-then-Exp so Exp is resident),
    masked suffix-sum scan (DVE), u=(S-1)*mask (DVE), a 1-element Ln that
    reads the exp output (cannot be hoisted) so the Ln table load overlaps
    the scan, Ln(x+1)+accumulate (ACT), masked-cterm accumulate (DVE).
  - Out = per-partition [16, 2] partials [ln_sum, mc_sum]; host computes
    numer = sum(c0 - c1) over partitions and cores, denom on host.
"""

import ml_dtypes
import numpy as np

import concourse.bacc as bacc
import concourse.bass as bass
import concourse.mybir as mybir
import concourse.tile as tile
from concourse._compat import with_exitstack
from concourse.bass_utils import run_bass_kernel_spmd

B, T, E, H = 128, 512, 128, 128
NCORES = 8
BPC = B // NCORES  # batch rows per core
NPAIR = BPC // 2

f32 = mybir.dt.float32
bf16 = mybir.dt.bfloat16

N_DUMMY = 6  # PE warm-up matmuls to lift the HAM clock gate


@with_exitstack
def _dli_body(ctx, tc):
    nc = tc.nc

    enc = nc.dram_tensor("enc", [BPC, T, E], f32, kind="ExternalInput").ap()
    m3 = nc.dram_tensor("mask3", [BPC, T], f32, kind="ExternalInput").ap()
    woh_d = nc.dram_tensor("woh", [E, BPC * BPC], bf16, kind="ExternalInput").ap()
    id_d = nc.dram_tensor("ident", [E, E], bf16, kind="ExternalInput").ap()
    out = nc.dram_tensor("out", [BPC, 2], f32, kind="ExternalOutput").ap()

    const_pool = ctx.enter_context(tc.tile_pool(name="const", bufs=1))
    chunk_pool = ctx.enter_context(tc.tile_pool(name="chunk", bufs=NPAIR))
    t4_pool = ctx.enter_context(tc.tile_pool(name="t4", bufs=6))
    tp_psum = ctx.enter_context(tc.tile_pool(name="tp_psum", bufs=4, space="PSUM"))
    ct_psum = ctx.enter_context(tc.tile_pool(name="ct_psum", bufs=1, space="PSUM"))
    dm_psum = ctx.enter_context(tc.tile_pool(name="dm_psum", bufs=1, space="PSUM"))
    sc_pool = ctx.enter_context(tc.tile_pool(name="scan", bufs=1))

    # --- enc via SWDGE cast-DMAs (fp32 HBM -> bf16 SBUF), 2 rows per DMA.
    # These are the ONLY gpsimd-queue instructions, so generation starts
    # right after the engine prologue.
    chunk_tiles = []
    for p in range(NPAIR):
        chunk = chunk_pool.tile([128, 2 * T], bf16)
        chunk_tiles.append(chunk)
        nc.gpsimd.dma_start(
            chunk[:].rearrange("p (r c e) -> p r c e", r=2, c=4),
            enc[2 * p : 2 * p + 2].rearrange("r (a c) e -> a r c e", c=4),
        )

    # --- host-precomputed constants on the Scalar HWDGE ring ---
    ident = const_pool.tile([128, 128], bf16)
    nc.scalar.dma_start(ident[:], id_d)
    woh = const_pool.tile([128, BPC * BPC], bf16)
    nc.scalar.dma_start(woh[:], woh_d)
    maskf = sc_pool.tile([BPC, T], f32)
    nc.scalar.dma_start(maskf[:], m3)
    mask3_rev = maskf[:, ::-1]

    # --- PE warm-up: real bf16 matmuls so HAM sees the PE busy ---
    ds = const_pool.tile([128, T], bf16)
    nc.vector.memset(ds[:].bitcast(mybir.dt.uint16), 0)
    dummy_ps = dm_psum.tile([128, T], f32)
    for _ in range(N_DUMMY):
        nc.tensor.matmul(
            dummy_ps[:, :], lhsT=ds[:, 0:128], rhs=ds[:], start=True, stop=True
        )

    # warm the ACT tables: Ln first, Exp last => the real Exp needs no table
    # load; the Ln reload is re-triggered right after the real Exp (below).
    warm = const_pool.tile([BPC, 1], f32)
    nc.vector.memset(warm[:], 0.0)
    nc.scalar.activation(warm[:], warm[:], mybir.ActivationFunctionType.Ln, bias=1.0)
    nc.scalar.activation(warm[:], warm[:], mybir.ActivationFunctionType.Exp)

    acc = sc_pool.tile([BPC, 2], f32)

    # --- main loop: per row, 4 bf16 transposes + copy + bf16 matvec ---
    cterm_ps = ct_psum.tile([BPC, T], f32)
    t4_tiles = [None] * BPC

    def emit_matvec(b):
        nc.tensor.matmul(
            cterm_ps[:, :],
            lhsT=woh[:, BPC * b : BPC * (b + 1)],
            rhs=t4_tiles[b][:],
            start=(b == 0),
            stop=(b == BPC - 1),
        )

    copy_engines = [
        lambda o, i: nc.vector.tensor_copy(o, i),
        lambda o, i: nc.scalar.copy(o, i),
    ]
    # DVE gets 10 copies, ACT gets 6 (ACT also owns exp/ln + table loads)
    copy_sel = [0, 1, 0, 0, 1, 0, 0, 1, 0, 1, 0, 0, 1, 0, 1, 0]
    SKEW = 3
    for b in range(BPC):
        chunk = chunk_tiles[b // 2]
        r = b % 2
        tp = tp_psum.tile([128, T], bf16)
        src = chunk[:].rearrange("p (r c e) -> p r c e", r=2, c=4)
        for j in range(4):
            nc.tensor.transpose(
                tp[:, 128 * j : 128 * (j + 1)], src[:, r, j], ident[:]
            )
        t4 = t4_pool.tile([128, T], bf16)
        t4_tiles[b] = t4
        copy_engines[copy_sel[b]](t4[:], tp[:])
        if b >= SKEW:
            emit_matvec(b - SKEW)
    for b in range(BPC - SKEW, BPC):
        emit_matvec(b)

    # un-permute + time-reverse view of the PSUM cterm: element i reads
    # cterm[b, 511 - i].
    cterm_rev = cterm_ps[:, :].rearrange("m (j p) -> m p j", j=4)[:, ::-1, ::-1]

    # E = exp(cterm)   (reversed-time coordinates, fused permute via the AP)
    e_sb = sc_pool.tile([BPC, T], f32)
    nc.scalar.activation(
        e_sb[:].rearrange("m (p j) -> m p j", j=4),
        cterm_rev,
        mybir.ActivationFunctionType.Exp,
    )
    # Re-trigger the Ln table load NOW so it overlaps the scan below.  Reads
    # e_sb so the scheduler cannot hoist it before the exp.
    lnwarm = sc_pool.tile([1, 1], f32)
    nc.scalar.activation(
        lnwarm[:], e_sb[0:1, 0:1], mybir.ActivationFunctionType.Ln, bias=1.0
    )

    # suffix sums with the mask folded in: state = (state + E[i]) * mask3_rev[i]
    s_sb = sc_pool.tile([BPC, T], f32)
    nc.vector.tensor_tensor_scan(
        s_sb[:], e_sb[:], mask3_rev, 0.0, mybir.AluOpType.add, mybir.AluOpType.mult
    )

    # u = (S - 1) * mask3; then ln(u + 1) = log(S) on valid, 0 on invalid
    u_sb = sc_pool.tile([BPC, T], f32)
    nc.vector.scalar_tensor_tensor(
        u_sb[:], s_sb[:], 1.0, mask3_rev,
        mybir.AluOpType.subtract, mybir.AluOpType.mult,
    )
    ln_sb = sc_pool.tile([BPC, T], f32)
    nc.scalar.activation(
        ln_sb[:], u_sb[:], mybir.ActivationFunctionType.Ln,
        bias=1.0, scale=1.0, accum_out=acc[:, 0:1],
    )
    # sum(mask3*cterm): order-free, so read the PSUM cterm unpermuted and the
    # mask through the matching permuted view.
    mc_sb = sc_pool.tile([BPC, T], f32)
    nc.vector.scalar_tensor_tensor(
        mc_sb[:].rearrange("m (j p) -> m j p", j=4),
        cterm_ps[:, :].rearrange("m (j p) -> m j p", j=4),
        0.0,
        maskf[:].rearrange("m (p j) -> m j p", j=4),
        mybir.AluOpType.add, mybir.AluOpType.mult, accum_out=acc[:, 1:2],
    )

    # out: per-partition partials; host computes sum(c0 - c1) / denom
    nc.sync.dma_start(out[:, :], acc[:])


_CACHED_NC = None


def _get_program():
    global _CACHED_NC
    if _CACHED_NC is None:
        nc = bacc.Bacc(
            "TRN2",
            target_bir_lowering=False,
            debug=False,
            enable_asserts=False,
        )
        with tile.TileContext(nc) as tc:
            _dli_body(tc)
        nc.compile()
        _CACHED_NC = nc
    return _CACHED_NC


def _make_in_maps(inputs):
    enc = np.ascontiguousarray(inputs["encoder_output"], dtype=np.float32)
    mask = np.asarray(inputs["mask"], dtype=np.int32)
    w_e = np.asarray(inputs["fc_w"], dtype=np.float32)[0, H:]

    mask3 = mask.astype(np.float32)
    mask3[:, 0:3] = 0.0

    woh = np.zeros((E, BPC * BPC), dtype=np.float32)
    for b in range(BPC):
        woh[:, BPC * b + b] = w_e
    woh = woh.astype(ml_dtypes.bfloat16)
    ident = np.eye(E, dtype=ml_dtypes.bfloat16)

    return [
        {
            "enc": np.ascontiguousarray(enc[i * BPC : (i + 1) * BPC]),
            "mask3": np.ascontiguousarray(mask3[i * BPC : (i + 1) * BPC]),
            "woh": woh,
            "ident": ident,
        }
        for i in range(NCORES)
    ]


def _denoms(inputs):
    mask = np.asarray(inputs["mask"], dtype=np.int64)
    lengths = mask.sum(axis=1)
    return lengths - 3


def kernel(**inputs) -> np.ndarray:
    nc = _get_program()
    res = run_bass_kernel_spmd(nc, _make_in_maps(inputs), list(range(NCORES)))
    numer = 0.0
    for r in res.results:
        o = np.asarray(r["out"], dtype=np.float64)
        numer += float(np.sum(o[:, 0] - o[:, 1]))
    denom = float(np.sum(_denoms(inputs)))
    return np.asarray(numer / denom, dtype=np.float32)
